# revision 1
# baseline (speedup 1.0000x reference)
"""Balanced CE loss + accuracy on 8 Trainium2 NeuronCores (Bass/Tile).

Reference computation (N = 16777216 elements):
    loss = -sum(where(t==1, 1.6*log(p), 0.4*log(1-p))) / N
    acc  = mean(round(p) == t)

Strategy (data-parallel over N, no collectives needed):
  Shard N across 8 cores; per core stream [128, C] sub-chunks so the
  DMA pipe never idles.  Single-variable encoding y = 1 - |p - t| folds
  both classes into one value:
    t==1 -> y = p,  t==0 -> y = 1-p
  so the per-element log term is ln(y) with class weight w = 1.2*t+0.4,
  and "correct" (round(p)==t) is exactly y >= 0.5  <=>  ln(y) >= -ln2.
  y is produced bf16 by ONE fused custom-DVE op (1 - maxx(p-t, t-p)),
  which kills both the separate z pass and the |z| pass and halves all
  downstream SBUF traffic (the kernel is SBUF-port-bound, ~36B/elem):
    DVE : y = 1-|p-t| bf16         (custom op, fp32 internal)
          S1[s] = sum (t>=1)*q     (stt, fused accum -> sum_{t=1} ln p)
          m = (q >= -ln2) bf16     (plain tensor_scalar, fast mode)
    ACT : q = Ln(y) bf16           (fused accum -> S[s] = sum ln(y))
    PE  : ones^T @ m -> one accumulating PSUM bank -> correct count
  bf16 y/q cost ~0.4% relative per element -- random sign, so the 16M
  sums keep ~5 digits; counting against the bf16-rounded -ln2 shifts
  acc by ~2e-4 relative.  Both are ~100x under the 2e-2 gate.
  Sub-chunks ramp [512,1024,1536,2048*5,1536,1024,512] so the pipeline
  fills fast and the last dependent chain is short; the custom-y ops
  are emitted two chunks ahead of the Ln-dependent tail ops so the DVE
  queue never head-of-line blocks on ACT.
  Host folds the [128, 2*NCH+1] partials in f64:
    loss = -(0.4*S + 1.2*S1)/N,  acc = C/N.
"""

import sys

if "/opt/trn_rl_repo" not in sys.path:
    sys.path.insert(0, "/opt/trn_rl_repo")

import numpy as np

import concourse.bass as bass
import concourse.bacc as bacc
import concourse.tile as tile
from concourse import mybir
from concourse.bass_utils import run_bass_kernel_spmd
import concourse.hw_specs as hw_specs
import concourse.dve_ops as dve_ops
from concourse.dve_ops import DveOp, OPS, CUSTOM_DVE_SPECS
from concourse.dve_spec import Spec, Src0, Src1, One, maxx, lower, _has_src1
from concourse.dve_uop import DveOpSpec

N_CORES = 8
N = 16777216
P = 128
SHARD = N // N_CORES          # 2097152 elements per core
COLS = SHARD // P             # 16384 columns per core
MMCOL = 512                   # matmul free-dim tile (one PSUM bank)

# chunk sizes: ramp up for fast pipeline fill, ramp down so the last
# y->q->{stt,mask} chain is short
SIZES = [256, 512, 1024, 1792] + [2048] * 5 + [1536, 512, 512]
assert sum(SIZES) == COLS
NCH = len(SIZES)

AF = mybir.ActivationFunctionType
OP = mybir.AluOpType
LN2 = 0.6931471805599453

_NC_CACHE = None

# The Tile list-scheduler orders engine streams from a CoreSim pass using
# TRN2Spec timings.  Its default DMA model (0.83 derate) believes input
# chunks land SLOWER than the DVE drains them, so it schedules each
# chunk's stt before the next chunk's y-op and the real machine then
# serializes Ln->stt->y->Ln cross-engine.  Believing a slightly faster
# DMA flips the order to y-first, which is what the real machine needs.
hw_specs.TRN2Spec.DMA_CYCLE = 1e9 / (400e9 / 128) / 1.05


def _ref_y(in0, in1, c0, c1, c2):
    return 1.0 - np.abs(in0.astype(np.float32) - in1.astype(np.float32))


def _register_custom_op():
    """Register y = 1 - |p - t| as a runtime custom-DVE op."""
    name = "Y_FROM_PT_ANT"
    if name in dve_ops._SUB_OPCODE_FOR_NAME:
        return next(op for op in OPS if op.name == name)
    spec = Spec(body=One - maxx(Src0 - Src1, Src1 - Src0), reference=_ref_y)
    row = max(dve_ops._SUB_OPCODE_FOR_NAME.values()) + 1
    assert row < 0x20
    dve_ops._SUB_OPCODE_FOR_NAME[name] = row
    shas = {}
    for ver in ("v3", "v4"):
        s = DveOpSpec(name=name, opcode=row, uops=lower(spec, ver=ver),
                      rd1_en=_has_src1(spec))
        shas[ver] = s.sha(ver)
    op = DveOp(name, spec, subdim=False, uops_sha=shas)
    OPS.append(op)
    CUSTOM_DVE_SPECS[name] = spec
    return op


def build_bass():
    """Build the single-core Bass program (SPMD across 8 cores)."""
    global _NC_CACHE
    if _NC_CACHE is not None:
        return _NC_CACHE

    y_op = _register_custom_op()

    nc = bacc.Bacc("TRN2", target_bir_lowering=False, debug=False)

    p_in = nc.dram_tensor("p_in", [SHARD], mybir.dt.float32, kind="ExternalInput").ap()
    t_in = nc.dram_tensor("t_in", [SHARD], mybir.dt.int32, kind="ExternalInput").ap()
    # acc cols: [s] sum sign(ln y + ln2) partials; [NCH+s] weighted log-sums
    acc = nc.dram_tensor("acc", [P, 3 * NCH], mybir.dt.float32, kind="ExternalOutput").ap()

    with tile.TileContext(nc) as tc:
        with (
            tc.tile_pool(name="io", bufs=8) as io_pool,
            tc.tile_pool(name="yp", bufs=6) as y_pool,
            tc.tile_pool(name="misc", bufs=1) as misc_pool,
            tc.tile_pool(name="psj", bufs=1, space=bass.MemorySpace.PSUM) as psum_pool,
        ):
            warm = misc_pool.tile([P, 1], mybir.dt.float32, tag="warm")
            ln2c = misc_pool.tile([P, 1], mybir.dt.float32, tag="ln2c")
            acc_a = misc_pool.tile([P, NCH], mybir.dt.float32, tag="acca")
            acc_v = misc_pool.tile([P, NCH], mybir.dt.float32, tag="accv")
            junk_s = psum_pool.tile([P, max(SIZES)], mybir.dt.float32, tag="js")
            junk_g = psum_pool.tile([P, max(SIZES)], mybir.dt.float32, tag="jg")

            MX = max(SIZES)
            offs = [sum(SIZES[:i]) * P for i in range(NCH)]
            tiles = {}

            def issue_front(s):
                """DMA chunk s and compute y_s (software-pipelined ahead)."""
                sz = SIZES[s]
                p_f = io_pool.tile([P, MX], mybir.dt.float32, tag="p")
                t_f = io_pool.tile([P, MX], mybir.dt.int32, tag="t")
                y_f = y_pool.tile([P, MX], mybir.dt.bfloat16, tag="y")
                p_t, t_t, y_t = p_f[:, 0:sz], t_f[:, 0:sz], y_f[:, 0:sz]
                off = offs[s]
                nc.sync.dma_start(
                    p_t, p_in[off : off + sz * P].rearrange("(p f) -> p f", p=P)
                )
                nc.sync.dma_start(
                    t_t, t_in[off : off + sz * P].rearrange("(p f) -> p f", p=P)
                )
                # y = 1 - |p - t|  (one fused DVE op, bf16 out)
                nc.vector._custom_dve(y_op, out=y_t, in0=p_t, in1=t_t)
                tiles[s] = (t_t, y_t)

            def issue_back(s):
                """Ln + reductions for chunk s."""
                sz = SIZES[s]
                t_t, y_t = tiles.pop(s)
                q_t = y_t
                # q = ln(y), in place over the y tile
                nc.scalar.activation(q_t, y_t, AF.Ln)
                # count: sign(q + ln2) = +-1, accum -> G[s]; correct = (G+n)/2
                nc.scalar.activation(junk_g[:, 0:sz], q_t, AF.Sign, bias=ln2c[:, 0:1],
                                     accum_out=acc_a[:, s : s + 1])
                # W[s] = sum (1.2*t + 0.4) * q  -- the full weighted log-sum
                nc.vector.affine_mul_reduce(junk_s[:, 0:sz], acc_v[:, s : s + 1],
                                            t_t, q_t, 1.2, 0.4)

            Z_AHEAD = NCH
            for s in range(NCH + Z_AHEAD):
                if s < NCH:
                    issue_front(s)
                if s == 1:
                    # ln2 bias tile + ACT Ln-table warm, off the entry
                    # barrier path so the first DMA fires sooner
                    nc.vector.memset(warm[:], 0.5)
                    nc.vector.memset(ln2c[:], LN2)
                    nc.scalar.activation(warm[:], warm[:], AF.Ln)
                if s - Z_AHEAD >= 0:
                    issue_back(s - Z_AHEAD)

            nc.sync.dma_start(acc[:, 0:NCH], acc_a[:])
            nc.sync.dma_start(acc[:, NCH : 2 * NCH], acc_v[:])

    nc.finalize()
    _NC_CACHE = nc
    return nc


def make_in_maps(input, target):
    inp = np.ascontiguousarray(np.asarray(input, dtype=np.float32)).reshape(
        N_CORES, SHARD
    )
    tgt = np.ascontiguousarray(np.asarray(target, dtype=np.int32)).reshape(
        N_CORES, SHARD
    )
    return [{"p_in": inp[c], "t_in": tgt[c]} for c in range(N_CORES)]


def combine(results):
    """Host-side unshard: reduce the 8 cores' partial sums -> (loss, acc)."""
    W = G = 0.0
    for r in results:
        aa = np.asarray(r["acc"], dtype=np.float64)
        G += aa[:, 0:NCH].sum()
        W += aa[:, NCH : 2 * NCH].sum()
    loss = -W / N
    acc = (G + N) / 2.0 / N
    return np.float32(loss), np.float32(acc)


def run_on_hw(input, target, **spmd_kwargs):
    nc = build_bass()
    in_maps = make_in_maps(input, target)
    return run_bass_kernel_spmd(nc, in_maps, list(range(N_CORES)), **spmd_kwargs)


def kernel(input, target):
    br = run_on_hw(input, target)
    return combine(br.results)



# revision 2
# speedup vs baseline: 1.0523x; 1.0523x over previous
"""Balanced CE loss + accuracy on 8 Trainium2 NeuronCores (Bass/Tile).

Reference computation (N = 16777216 elements):
    loss = -sum(where(t==1, 1.6*log(p), 0.4*log(1-p))) / N
    acc  = mean(round(p) == t)

Strategy (data-parallel over N, no collectives needed):
  Shard N across 8 cores; per core stream [128, C] chunks.

  Weight-in-the-log trick: w = 0.4+1.2t = 0.4*(1+3t), so
      sum w*ln(y) = 0.4 * sum ln(y^(1+3t)),   y = 1-|p-t| = |p+t-1|.
  One custom DVE op computes u = y^(1+3t) (= y if t==0 else y^4) in
  8 ALU slices:  c=1-t; d=p-c; y=max(d,c-p); u=min(y, y^4+c).
  y >= 1e-6 so u >= 1e-24 -- comfortably inside bf16 normal range.
  ONE ACT pass Ln(u) with fused accumulation then yields the whole
  per-chunk weighted log-sum; no second log pass, no cross term.

  A second custom DVE op counts correct predictions exactly in fp32:
      m = ((p-(1-t))^2 >= 0.25)  ==  (y >= 0.5)  ==  (round(p)==t),
  with fused accum -> per-chunk count (integers, exact in fp32).

  Engine budget per core (2M elems, chunked by 2048 cols):
      DMA  ~43.6us (16.8 MB at ~410 GB/s streaming)   <- bottleneck
      DVE  ~37us  (2 passes at 1x: U-op + M-op)
      ACT  ~21us  (1 Ln pass + accumulator reads)
  All DVE reduce/accum paths run at 1x regardless of dtype (measured),
  so the win over the 4-pass baseline comes from needing only 3 passes
  total and a shallow dependency graph: DMA -> {U,M} -> Ln.

  Host folds the [128, 2*NCH] partials in f64:
    loss = -0.4*sum(W)/N,  acc = sum(C)/N  (count is exact).
"""

import sys

if "/opt/trn_rl_repo" not in sys.path:
    sys.path.insert(0, "/opt/trn_rl_repo")

import numpy as np

import concourse.bass as bass
import concourse.bacc as bacc
import concourse.tile as tile
from concourse import mybir
from concourse.bass_utils import run_bass_kernel_spmd
import concourse.hw_specs as hw_specs
import concourse.dve_ops as dve_ops
from concourse.dve_ops import DveOp, OPS, CUSTOM_DVE_SPECS
from concourse.dve_spec import (
    Spec, Src0, Src1, One, C0, sq, maxx, minn, lower, AluOp, _has_src1,
)
from concourse.dve_uop import DveOpSpec

N_CORES = 8
N = 16777216
P = 128
SHARD = N // N_CORES          # 2097152 elements per core
COLS = SHARD // P             # 16384 columns per core

# chunk sizes: small first chunk so compute starts early, small last
# chunk so the final DMA->U->Ln chain is short
SIZES = [512, 1024, 2048, 2048, 2048, 2048, 2048, 2048, 1536, 768, 256]
assert sum(SIZES) == COLS
NCH = len(SIZES)
MX = max(SIZES)

AF = mybir.ActivationFunctionType

_NC_CACHE = None

# Bias the Tile list-scheduler's DMA model slightly fast so it orders
# engine streams DMA-first (see baseline notes); harmless otherwise.
hw_specs.TRN2Spec.DMA_CYCLE = 1e9 / (400e9 / 128) / 1.05


def _ref_u(in0, in1, c0, c1, c2):
    t = in1.astype(np.float32)
    y = np.abs(in0.astype(np.float32) + t - 1.0)
    return np.minimum(y, np.square(np.square(y)) + (1.0 - t)).astype(np.float32)


def _ref_m(in0, in1, c0, c1, c2):
    t = in1.astype(np.float32)
    d = in0.astype(np.float32) + t - 1.0
    out = (d * d >= np.float32(c0)).astype(np.float32)
    acc = out.reshape(out.shape[0], -1).sum(axis=-1, keepdims=True)
    return out, acc


def _register_op(name, spec):
    if name in dve_ops._SUB_OPCODE_FOR_NAME:
        return next(op for op in OPS if op.name == name)
    row = max(dve_ops._SUB_OPCODE_FOR_NAME.values()) + 1
    assert row < 0x20
    dve_ops._SUB_OPCODE_FOR_NAME[name] = row
    shas = {}
    for ver in ("v3", "v4"):
        s = DveOpSpec(name=name, opcode=row, uops=lower(spec, ver=ver),
                      rd1_en=_has_src1(spec))
        shas[ver] = s.sha(ver)
    op = DveOp(name, spec, subdim=False, uops_sha=shas)
    OPS.append(op)
    CUSTOM_DVE_SPECS[name] = spec
    return op


def _register_custom_ops():
    # U: u = y^(1+3t),  y = |p+t-1|
    c = One - Src1
    d = Src0 - c
    y = maxx(d, c - Src0)
    u_body = minn(y, sq(sq(y)) + c)
    u_op = _register_op("U_WPOW_ANT", Spec(body=u_body, reference=_ref_u))
    # M: m = ((p-(1-t))^2 >= s0), accum add -> exact correct count
    m_body = sq(Src0 - (One - Src1)) >= C0
    m_op = _register_op(
        "M_COUNT_ANT", Spec(body=m_body, accum=AluOp.ADD, reference=_ref_m)
    )
    return u_op, m_op


def build_bass():
    """Build the single-core Bass program (SPMD across 8 cores)."""
    global _NC_CACHE
    if _NC_CACHE is not None:
        return _NC_CACHE

    u_op, m_op = _register_custom_ops()

    nc = bacc.Bacc("TRN2", target_bir_lowering=False, debug=False)

    p_in = nc.dram_tensor("p_in", [SHARD], mybir.dt.float32, kind="ExternalInput").ap()
    t_in = nc.dram_tensor("t_in", [SHARD], mybir.dt.int32, kind="ExternalInput").ap()
    # acc cols: [s] = sum ln(u) partials; [NCH+s] = correct counts
    acc = nc.dram_tensor("acc", [P, 2 * NCH], mybir.dt.float32, kind="ExternalOutput").ap()

    with tile.TileContext(nc) as tc:
        with (
            tc.tile_pool(name="io", bufs=5) as io_pool,
            tc.tile_pool(name="up", bufs=4) as u_pool,
            tc.tile_pool(name="misc", bufs=1) as misc_pool,
            tc.tile_pool(name="psj", bufs=1, space=bass.MemorySpace.PSUM) as psum_pool,
        ):
            warm = misc_pool.tile([P, 1], mybir.dt.float32, tag="warm")
            acc_w = misc_pool.tile([P, NCH], mybir.dt.float32, tag="accw")
            acc_c = misc_pool.tile([P, NCH], mybir.dt.float32, tag="accc")
            junk_q = psum_pool.tile([P, MX], mybir.dt.float32, tag="jq")
            junk_m = psum_pool.tile([P, MX], mybir.dt.float32, tag="jm")

            offs = [sum(SIZES[:i]) * P for i in range(NCH)]
            tiles = {}

            def issue_dma(s):
                sz = SIZES[s]
                p_f = io_pool.tile([P, MX], mybir.dt.float32, tag="p")
                t_f = io_pool.tile([P, MX], mybir.dt.int32, tag="t")
                p_t, t_t = p_f[:, 0:sz], t_f[:, 0:sz]
                off = offs[s]
                nc.sync.dma_start(
                    p_t, p_in[off : off + sz * P].rearrange("(p f) -> p f", p=P)
                )
                nc.sync.dma_start(
                    t_t, t_in[off : off + sz * P].rearrange("(p f) -> p f", p=P)
                )
                tiles[s] = (p_t, t_t)

            def issue_compute(s):
                sz = SIZES[s]
                p_t, t_t = tiles.pop(s)
                u_f = u_pool.tile([P, MX], mybir.dt.bfloat16, tag="u")
                u_t = u_f[:, 0:sz]
                # u = y^(1+3t)  (one fused DVE op)
                nc.vector._custom_dve(u_op, out=u_t, in0=p_t, in1=t_t)
                # exact correct-count with fused accum
                nc.vector._custom_dve(
                    m_op, out=junk_m[:, 0:sz], in0=p_t, in1=t_t, s0=0.25,
                    accum_out=acc_c[:, s : s + 1],
                )
                # weighted log-sum in one ACT pass: accum(ln u) = W_s/0.4
                nc.scalar.activation(
                    junk_q[:, 0:sz], u_t, AF.Ln, accum_out=acc_w[:, s : s + 1]
                )

            AHEAD = 2
            for s in range(NCH + AHEAD):
                if s < NCH:
                    issue_dma(s)
                if s == 1:
                    # warm the ACT Ln table off the critical path
                    nc.vector.memset(warm[:], 0.5)
                    nc.scalar.activation(warm[:], warm[:], AF.Ln)
                if s - AHEAD >= 0:
                    issue_compute(s - AHEAD)

            nc.sync.dma_start(acc[:, 0:NCH], acc_w[:])
            nc.sync.dma_start(acc[:, NCH : 2 * NCH], acc_c[:])

    nc.finalize()
    _NC_CACHE = nc
    return nc


def make_in_maps(input, target):
    inp = np.ascontiguousarray(np.asarray(input, dtype=np.float32)).reshape(
        N_CORES, SHARD
    )
    tgt = np.ascontiguousarray(np.asarray(target, dtype=np.int32)).reshape(
        N_CORES, SHARD
    )
    return [{"p_in": inp[c], "t_in": tgt[c]} for c in range(N_CORES)]


def combine(results):
    """Host-side unshard: fold the 8 cores' partials -> (loss, acc)."""
    W = C = 0.0
    for r in results:
        aa = np.asarray(r["acc"], dtype=np.float64)
        W += aa[:, 0:NCH].sum()
        C += aa[:, NCH : 2 * NCH].sum()
    loss = -0.4 * W / N
    acc = C / N
    return np.float32(loss), np.float32(acc)


def run_on_hw(input, target, **spmd_kwargs):
    nc = build_bass()
    in_maps = make_in_maps(input, target)
    return run_bass_kernel_spmd(nc, in_maps, list(range(N_CORES)), **spmd_kwargs)


def kernel(input, target):
    br = run_on_hw(input, target)
    return combine(br.results)
